# revision 2
# baseline (speedup 1.0000x reference)
"""Trainium2 Bass kernel for nn_InvariantMapping (topk_masking).

Math: score[b,n] = sum_{d,d'} fxpar[b,d,n] * G[b,d,d',n] * fypar[b,d',n]
with G = sum_c fx*fy, and fxpar derived from the channel mean. Softmax is
monotonic so top-k needs only the raw scores. The device computes, in a
single pass over fx/fy (memory-bound), 15 per-point channel reductions:
Sx_d, Sy_d (channel sums) and the 9 Gram components G_{dd'}. The tiny
finalize (norms/eps/score), top-8 selection, and gather run on the host.

Sharding: data-parallel over batch, 2 batches per core on 8 cores.

Device layout per (batch, n-tile of 512):
 - two c-groups of 128 channels on partitions
 - DVE forms the 9 products fx_d*fy_d' per group
 - reductions via matmul with a stationary ones[128,32]: out rows are the
   channel sum replicated 32x; 3 components per PSUM bank at partition
   bases {0,32,64}, 15 components over 5 banks (two psum tiles, 3+2 banks)
 - per component: group0 matmul (start=True), group1 (stop=True)
   back-to-back (a bank-wide has_written clear makes interleaving unsafe)
 - ACT evicts PSUM->SBUF strip; DMA-out reads strip rows {0,32,64} with a
   partition-stride-32 access pattern (tiny output: 15 f32 per point)
"""
import sys

sys.path.insert(0, "/opt/trn_rl_repo")

import numpy as np

B, C, D, NPTS = 16, 256, 3, 16384
NCORES = 8
BPC = B // NCORES  # batches per core
NT = 512  # n-tile (one PSUM bank of fp32)
NTILES = NPTS // NT
EPS = 1e-6

_CACHE = {}


def _build_nc():
    import concourse.bacc as bacc
    import concourse.bass as bass
    import concourse.mybir as mybir
    import concourse.tile as tile

    f32 = mybir.dt.float32
    nc = bacc.Bacc()
    fxs = nc.dram_tensor("fxs", [BPC, C, D, NPTS], f32, kind="ExternalInput")
    fys = nc.dram_tensor("fys", [BPC, C, D, NPTS], f32, kind="ExternalInput")
    comps = nc.dram_tensor(
        "comps", [BPC, NTILES, 3, 5, NT], f32, kind="ExternalOutput"
    )

    with tile.TileContext(nc) as tc:
        with (
            tc.tile_pool(name="io", bufs=4) as io,
            tc.tile_pool(name="onesp", bufs=1) as onesp,
            tc.tile_pool(name="prod", bufs=8) as prodp,
            tc.tile_pool(name="psA", bufs=1, space="PSUM") as psa,
            tc.tile_pool(name="psB", bufs=1, space="PSUM") as psb,
            tc.tile_pool(name="strip", bufs=2) as stripp,
        ):
            ones32 = onesp.tile([128, 32], f32)
            nc.vector.memset(ones32, 1.0)

            for b in range(BPC):
                for t in range(NTILES):
                    n0 = NT * t
                    xt, yt = [], []
                    for g in range(2):
                        c0 = 128 * g
                        xg = io.tile([128, D, NT], f32, tag="fx")
                        yg = io.tile([128, D, NT], f32, tag="fy")
                        nc.sync.dma_start(
                            out=xg, in_=fxs[b, c0 : c0 + 128, :, n0 : n0 + NT]
                        )
                        nc.sync.dma_start(
                            out=yg, in_=fys[b, c0 : c0 + 128, :, n0 : n0 + NT]
                        )
                        xt.append(xg)
                        yt.append(yg)

                    # 9 Gram products per c-group
                    pr = {}
                    for g in range(2):
                        for d in range(D):
                            p = prodp.tile([128, D, NT], f32, tag="pr")
                            for dp in range(D):
                                nc.vector.tensor_mul(
                                    p[:, dp, :], xt[g][:, d, :], yt[g][:, dp, :]
                                )
                            pr[(g, d)] = p

                    pa = psa.tile([96, 3, NT], f32)
                    pb = psb.tile([96, 2, NT], f32)
                    for k in range(15):
                        j, r = k // 3, 32 * (k % 3)
                        out = pa[r : r + 32, j, :] if j < 3 else pb[r : r + 32, j - 3, :]
                        for g in range(2):
                            if k < 3:
                                rhs = xt[g][:, k, :]
                            elif k < 6:
                                rhs = yt[g][:, k - 3, :]
                            else:
                                m = k - 6
                                rhs = pr[(g, m // 3)][:, m % 3, :]
                            nc.tensor.matmul(
                                out, ones32, rhs, start=(g == 0), stop=(g == 1)
                            )

                    st = stripp.tile([96, 5, NT], f32)
                    nc.scalar.copy(out=st[:, 0:3, :], in_=pa)
                    nc.scalar.copy(out=st[:, 3:5, :], in_=pb)
                    strided = bass.AP(
                        tensor=st.tensor,
                        offset=st.offset,
                        ap=[[32 * st.ap[0][0], 3]] + list(st.ap[1:]),
                    )
                    nc.sync.dma_start(out=comps[b, t], in_=strided)
    nc.finalize()
    return nc


def _get_nc():
    if "nc" not in _CACHE:
        _CACHE["nc"] = _build_nc()
    return _CACHE["nc"]


def _run_device(fx, fy, trace=False):
    from concourse.bass_utils import run_bass_kernel_spmd

    nc = _get_nc()
    in_maps = []
    for i in range(NCORES):
        sl = slice(BPC * i, BPC * (i + 1))
        in_maps.append(
            {
                "fxs": np.ascontiguousarray(fx[sl]),
                "fys": np.ascontiguousarray(fy[sl]),
            }
        )
    res = run_bass_kernel_spmd(
        nc, in_maps, core_ids=list(range(NCORES)), trace=trace
    )
    out = np.stack([r["comps"] for r in res.results])  # [8, BPC, NTILES, 3, 5, NT]
    return out, res


def _scores_from_comps(out):
    # out: [8, BPC, NTILES, 3(rows r), 5(banks j), NT]; comp k = 3*j + r
    a = out.astype(np.float64)
    a = a.transpose(0, 1, 4, 3, 2, 5)  # [8, BPC, j, r, NTILES, NT]
    a = a.reshape(NCORES * BPC, 15, NPTS)  # comp k = 3*j + r ordering
    Sx = a[:, 0:3]  # [B, 3, n]
    Sy = a[:, 3:6]
    G = a[:, 6:15].reshape(B, 3, 3, NPTS)
    mx = Sx / C
    my = Sy / C
    nx = np.sqrt((mx**2).sum(1, keepdims=True)) + EPS
    ny = np.sqrt((my**2).sum(1, keepdims=True)) + EPS
    px = mx / nx
    py = my / ny
    score = np.einsum("bdn,bden,ben->bn", px, G, py)
    return score


def kernel(fx, fy, topk):
    fx = np.asarray(fx, dtype=np.float32)
    fy = np.asarray(fy, dtype=np.float32)
    kk = B // int(topk)
    out, _ = _run_device(fx, fy)
    score = _scores_from_comps(out)
    # jax.lax.top_k order: descending value, ties -> lower index (stable)
    idx = np.argsort(-score, axis=1, kind="stable")[:, :kk].astype(np.int32)
    idxe = idx[:, None, None, :]
    fx_sel = np.take_along_axis(fx, idxe, axis=3)
    fy_sel = np.take_along_axis(fy, idxe, axis=3)
    return (fx_sel, fy_sel)
